# revision 16
# baseline (speedup 1.0000x reference)
"""Non-overlapping Conv1d (patch GEMM) on 8 TRN2 NeuronCores.

Problem (hardcoded shapes):
  x:      (16, 64, 65536) f32
  weight: (64, 64, 1, 4)  f32
  bias:   (1, 64, 16384)  f32
  y[b,o,n] = sum_{c,k} x[b,c,4n+k] * w[o,c,k] / 8 + 0.1*bias[0,o,n]
  -> y: (16, 64, 16384) f32

Sharding: spatial (d) dim split 8 ways -- each core gets all 16 batches
over a 8192-wide d slice (2048 patches). Bias is sharded too, so every
HBM byte is read exactly once system-wide (~41 MiB/core traffic).

Per-core kernel: for each batch pair, DMA a [128, 8192] tile (2 batches
x 64 channels). Contraction (cin=64, k=4) is 4 PSUM-accumulated K=64
matmuls per 512-patch chunk; batch 0 runs on PE quadrant (0,0) ->
psum[0:64], batch 1 on quadrant (64,64) -> psum[64:128] (concurrent
row/col tiles). Weights are pre-transposed/scaled on the host (w/8);
bias is pre-scaled (0.1*bias) and added during the PSUM->SBUF drain.
Matmul inputs are bitcast to float32r (full-rate fp32 PE mode).
"""

import numpy as np

BS, CIN, D = 16, 64, 65536
OUT_C = 64
K = 4
OUT_DIM = D // K          # 16384 patches
N_CORES = 8
D_SH = D // N_CORES       # 8192 d-elems per core
N_SH = OUT_DIM // N_CORES  # 2048 patches per core
CHUNK = 512               # psum bank = 512 f32
D_T = 4096                # d-elems per x tile (half a batch-pair shard)
N_T = D_T // K            # 1024 patches per tile
N_CHUNKS = N_T // CHUNK   # 2
MM_DT = "float32r"        # PE matmul dtype ("float32r" or "float32")

_CACHE = {}


def _build_module():
    import concourse.bass as bass
    import concourse.tile as tile
    from concourse import bacc, mybir

    f32 = mybir.dt.float32
    mm_dt = getattr(mybir.dt, MM_DT)

    nc = bacc.Bacc("TRN2", target_bir_lowering=False, debug=False)

    x = nc.dram_tensor("x", (BS, CIN, D_SH), mm_dt, kind="ExternalInput").ap()
    wt = nc.dram_tensor("wt", (128, K * OUT_C), mm_dt, kind="ExternalInput").ap()
    bias = nc.dram_tensor("bias", (OUT_C, N_SH), f32, kind="ExternalInput").ap()
    y = nc.dram_tensor("y", (BS, OUT_C, N_SH), f32, kind="ExternalOutput").ap()

    with tile.TileContext(nc) as tc:
        with (
            tc.tile_pool(name="const", bufs=1) as const_pool,
            tc.tile_pool(name="xin", bufs=6) as xpool,
            tc.tile_pool(name="out", bufs=4) as opool,
            tc.tile_pool(name="psum", bufs=4, space=bass.MemorySpace.PSUM) as pspool,
        ):
            w_sb = const_pool.tile([128, K * OUT_C], mm_dt)
            nc.sync.dma_start(w_sb[:], wt[:])
            b_sb = const_pool.tile([64, N_SH], f32)
            nc.sync.dma_start(b_sb[:], bias[:])

            for pt in range(BS // 2 * (D_SH // D_T)):
                p, t = divmod(pt, D_SH // D_T)
                xt = xpool.tile([128, D_T], mm_dt)
                nc.sync.dma_start(
                    xt[:],
                    x[2 * p : 2 * p + 2, :, t * D_T : (t + 1) * D_T].rearrange(
                        "b c d -> (b c) d"
                    ),
                )
                # [128, k, n] strided view of the patch layout
                xr = xt[:].rearrange("p (n k) -> p k n", k=K)
                ot = opool.tile([128, N_T], f32)
                for j in range(N_CHUNKS):
                    # one psum bank per batch half; fp32r matmuls must write
                    # PSUM at partition 0, so the halves use PE row-tiles
                    # (0,0)/(64,0) into separate banks
                    pss = [
                        pspool.tile([64, CHUNK], f32, name=f"ps{h}", tag=f"ps{h}")
                        for h in (0, 1)
                    ]
                    for k in range(K):
                        for h in (0, 1):
                            sl = slice(64 * h, 64 * (h + 1))
                            nc.tensor.matmul(
                                pss[h][:, :],
                                w_sb[sl, 64 * k : 64 * (k + 1)],
                                xr[sl, k, CHUNK * j : CHUNK * (j + 1)],
                                start=(k == 0),
                                stop=(k == K - 1),
                            )
                    for h in (0, 1):
                        sl = slice(64 * h, 64 * (h + 1))
                        nc.vector.tensor_add(
                            ot[sl, CHUNK * j : CHUNK * (j + 1)],
                            pss[h][:, :],
                            b_sb[:, t * N_T + CHUNK * j : t * N_T + CHUNK * (j + 1)],
                        )
                nc.sync.dma_start(
                    y[2 * p : 2 * p + 2, :, t * N_T : (t + 1) * N_T].rearrange(
                        "b c d -> (b c) d"
                    ),
                    ot[:],
                )
    nc.compile()
    return nc


def get_module():
    if "nc" not in _CACHE:
        _CACHE["nc"] = _build_module()
    return _CACHE["nc"]


def make_in_maps(x, weight, bias):
    """Host-side prep: shard x/bias over d, pre-transpose+scale weight."""
    x = np.asarray(x, dtype=np.float32)
    weight = np.asarray(weight, dtype=np.float32)
    bias = np.asarray(bias, dtype=np.float32)

    # wt[h*64+c, k*64+o] = weight[o, c, 0, k] / 8  (fold 1/sqrt(cin))
    wtb = (weight[:, :, 0, :] / np.sqrt(CIN)).transpose(1, 2, 0).reshape(CIN, K * OUT_C)
    wt = np.empty((128, K * OUT_C), np.float32)
    wt[0:64] = wtb
    wt[64:128] = wtb

    bias_p = (bias[0] * 0.1).astype(np.float32)  # (64, 16384)

    in_maps = []
    for c in range(N_CORES):
        in_maps.append(
            {
                "x": np.ascontiguousarray(x[:, :, c * D_SH : (c + 1) * D_SH]),
                "wt": wt,
                "bias": np.ascontiguousarray(bias_p[:, c * N_SH : (c + 1) * N_SH]),
            }
        )
    return in_maps


def run(x, weight, bias, **spmd_kwargs):
    """Run on 8 cores; returns (full y, BassKernelResults)."""
    from concourse import bass_utils

    nc = get_module()
    in_maps = make_in_maps(x, weight, bias)
    res = bass_utils.run_bass_kernel_spmd(
        nc, in_maps, core_ids=list(range(N_CORES)), **spmd_kwargs
    )
    y = np.empty((BS, OUT_C, OUT_DIM), np.float32)
    for c in range(N_CORES):
        y[:, :, c * N_SH : (c + 1) * N_SH] = res.results[c]["y"]
    return y, res


def kernel(x, weight, bias):
    y, _ = run(x, weight, bias)
    return y


# revision 18
# speedup vs baseline: 1.1660x; 1.1660x over previous
"""Non-overlapping Conv1d (patch GEMM) on 8 TRN2 NeuronCores.

Problem (hardcoded shapes):
  x:      (16, 64, 65536) f32
  weight: (64, 64, 1, 4)  f32
  bias:   (1, 64, 16384)  f32
  y[b,o,n] = sum_{c,k} x[b,c,4n+k] * w[o,c,k] / 8 + 0.1*bias[0,o,n]
  -> y: (16, 64, 16384) f32

Sharding: spatial (d) dim split 8 ways -- each core gets all 16 batches
over a 8192-wide d slice (2048 patches). Bias is sharded too, so every
HBM byte is read exactly once system-wide (~41 MiB/core traffic).

Per-core kernel: for each batch pair, DMA a [128, 8192] tile (2 batches
x 64 channels). Contraction (cin=64, k=4) is 4 PSUM-accumulated K=64
matmuls per 512-patch chunk; batch 0 runs on PE quadrant (0,0) ->
psum[0:64], batch 1 on quadrant (64,64) -> psum[64:128] (concurrent
row/col tiles). Weights are pre-transposed/scaled on the host (w/8);
bias is pre-scaled (0.1*bias) and added during the PSUM->SBUF drain.
Matmul inputs are bitcast to float32r (full-rate fp32 PE mode).
"""

import numpy as np

BS, CIN, D = 16, 64, 65536
OUT_C = 64
K = 4
OUT_DIM = D // K          # 16384 patches
N_CORES = 8
D_SH = D // N_CORES       # 8192 d-elems per core
N_SH = OUT_DIM // N_CORES  # 2048 patches per core
CHUNK = 512               # psum bank = 512 f32
D_T = 8192                # d-elems per x tile (one batch-pair shard)
N_T = D_T // K            # 1024 patches per tile
N_CHUNKS = N_T // CHUNK   # 2
MM_DT = "float32r"        # PE matmul dtype ("float32r" or "float32")

_CACHE = {}


def _build_module():
    import concourse.bass as bass
    import concourse.tile as tile
    from concourse import bacc, mybir

    f32 = mybir.dt.float32
    mm_dt = getattr(mybir.dt, MM_DT)

    nc = bacc.Bacc("TRN2", target_bir_lowering=False, debug=False)

    x = nc.dram_tensor("x", (BS, CIN, D_SH), mm_dt, kind="ExternalInput").ap()
    wt = nc.dram_tensor("wt", (128, K * OUT_C), mm_dt, kind="ExternalInput").ap()
    bias = nc.dram_tensor("bias", (OUT_C, N_SH), f32, kind="ExternalInput").ap()
    y = nc.dram_tensor("y", (BS, OUT_C, N_SH), f32, kind="ExternalOutput").ap()

    with tile.TileContext(nc) as tc:
        with (
            tc.tile_pool(name="const", bufs=1) as const_pool,
            tc.tile_pool(name="xin", bufs=4) as xpool,
            tc.tile_pool(name="out", bufs=3) as opool,
            tc.tile_pool(name="psum", bufs=4, space=bass.MemorySpace.PSUM) as pspool,
        ):
            w_sb = const_pool.tile([128, K * OUT_C], mm_dt)
            nc.sync.dma_start(w_sb[:], wt[:])
            b_sb = const_pool.tile([64, N_SH], f32)
            nc.sync.dma_start(b_sb[:], bias[:])

            for pt in range(BS // 2 * (D_SH // D_T)):
                p, t = divmod(pt, D_SH // D_T)
                xt = xpool.tile([128, D_T], mm_dt)
                nc.sync.dma_start(
                    xt[:],
                    x[2 * p : 2 * p + 2, :, t * D_T : (t + 1) * D_T].rearrange(
                        "b c d -> (b c) d"
                    ),
                )
                # [128, k, n] strided view of the patch layout
                xr = xt[:].rearrange("p (n k) -> p k n", k=K)
                ot = opool.tile([128, N_T], f32)
                for j in range(N_CHUNKS):
                    # one psum bank per batch half; fp32r matmuls must write
                    # PSUM at partition 0, so the halves use PE row-tiles
                    # (0,0)/(64,0) into separate banks
                    pss = [
                        pspool.tile([64, CHUNK], f32, name=f"ps{h}", tag=f"ps{h}")
                        for h in (0, 1)
                    ]
                    for k in range(K):
                        for h in (0, 1):
                            sl = slice(64 * h, 64 * (h + 1))
                            nc.tensor.matmul(
                                pss[h][:, :],
                                w_sb[sl, 64 * k : 64 * (k + 1)],
                                xr[sl, k, CHUNK * j : CHUNK * (j + 1)],
                                start=(k == 0),
                                stop=(k == K - 1),
                            )
                    for h in (0, 1):
                        sl = slice(64 * h, 64 * (h + 1))
                        nc.vector.tensor_add(
                            ot[sl, CHUNK * j : CHUNK * (j + 1)],
                            pss[h][:, :],
                            b_sb[:, t * N_T + CHUNK * j : t * N_T + CHUNK * (j + 1)],
                        )
                nc.sync.dma_start(
                    y[2 * p : 2 * p + 2, :, t * N_T : (t + 1) * N_T].rearrange(
                        "b c d -> (b c) d"
                    ),
                    ot[:],
                )
    nc.compile()
    return nc


def get_module():
    if "nc" not in _CACHE:
        _CACHE["nc"] = _build_module()
    return _CACHE["nc"]


def make_in_maps(x, weight, bias):
    """Host-side prep: shard x/bias over d, pre-transpose+scale weight."""
    x = np.asarray(x, dtype=np.float32)
    weight = np.asarray(weight, dtype=np.float32)
    bias = np.asarray(bias, dtype=np.float32)

    # wt[h*64+c, k*64+o] = weight[o, c, 0, k] / 8  (fold 1/sqrt(cin))
    wtb = (weight[:, :, 0, :] / np.sqrt(CIN)).transpose(1, 2, 0).reshape(CIN, K * OUT_C)
    wt = np.empty((128, K * OUT_C), np.float32)
    wt[0:64] = wtb
    wt[64:128] = wtb

    bias_p = (bias[0] * 0.1).astype(np.float32)  # (64, 16384)

    in_maps = []
    for c in range(N_CORES):
        in_maps.append(
            {
                "x": np.ascontiguousarray(x[:, :, c * D_SH : (c + 1) * D_SH]),
                "wt": wt,
                "bias": np.ascontiguousarray(bias_p[:, c * N_SH : (c + 1) * N_SH]),
            }
        )
    return in_maps


def run(x, weight, bias, **spmd_kwargs):
    """Run on 8 cores; returns (full y, BassKernelResults)."""
    from concourse import bass_utils

    nc = get_module()
    in_maps = make_in_maps(x, weight, bias)
    res = bass_utils.run_bass_kernel_spmd(
        nc, in_maps, core_ids=list(range(N_CORES)), **spmd_kwargs
    )
    y = np.empty((BS, OUT_C, OUT_DIM), np.float32)
    for c in range(N_CORES):
        y[:, :, c * N_SH : (c + 1) * N_SH] = res.results[c]["y"]
    return y, res


def kernel(x, weight, bias):
    y, _ = run(x, weight, bias)
    return y


# revision 24
# speedup vs baseline: 1.1840x; 1.0154x over previous
"""Non-overlapping Conv1d (patch GEMM) on 8 TRN2 NeuronCores.

Problem (hardcoded shapes):
  x:      (16, 64, 65536) f32
  weight: (64, 64, 1, 4)  f32
  bias:   (1, 64, 16384)  f32
  y[b,o,n] = sum_{c,k} x[b,c,4n+k] * w[o,c,k] / 8 + 0.1*bias[0,o,n]
  -> y: (16, 64, 16384) f32

Sharding: spatial (d) dim split 8 ways -- each core gets all 16 batches
over a 8192-wide d slice (2048 patches). Bias is sharded too, so every
HBM byte is read exactly once system-wide (~41 MiB/core traffic).

Per-core kernel: for each batch pair, DMA a [128, 8192] tile (2 batches
x 64 channels). Contraction (cin=64, k=4) is 4 PSUM-accumulated K=64
matmuls per 512-patch chunk; batch 0 runs on PE quadrant (0,0) ->
psum[0:64], batch 1 on quadrant (64,64) -> psum[64:128] (concurrent
row/col tiles). Weights are pre-transposed/scaled on the host (w/8);
bias is pre-scaled (0.1*bias) and added during the PSUM->SBUF drain.
Matmul inputs are bitcast to float32r (full-rate fp32 PE mode).
"""

import numpy as np

BS, CIN, D = 16, 64, 65536
OUT_C = 64
K = 4
OUT_DIM = D // K          # 16384 patches
N_CORES = 8
D_SH = D // N_CORES       # 8192 d-elems per core
N_SH = OUT_DIM // N_CORES  # 2048 patches per core
CHUNK = 512               # psum bank = 512 f32
D_T = 8192                # d-elems per x tile (one batch-pair shard)
N_T = D_T // K            # 1024 patches per tile
N_CHUNKS = N_T // CHUNK   # 2
MM_DT = "float32r"        # PE matmul dtype ("float32r" or "float32")

_CACHE = {}


def _build_module():
    import concourse.bass as bass
    import concourse.tile as tile
    from concourse import bacc, mybir

    f32 = mybir.dt.float32
    mm_dt = getattr(mybir.dt, MM_DT)

    nc = bacc.Bacc("TRN2", target_bir_lowering=False, debug=False)

    x = nc.dram_tensor("x", (BS, CIN, D_SH), mm_dt, kind="ExternalInput").ap()
    wt = nc.dram_tensor("wt", (128, K * OUT_C), mm_dt, kind="ExternalInput").ap()
    bias = nc.dram_tensor("bias", (OUT_C, N_SH), f32, kind="ExternalInput").ap()
    y = nc.dram_tensor("y", (BS, OUT_C, N_SH), f32, kind="ExternalOutput").ap()

    with tile.TileContext(nc) as tc:
        with (
            tc.tile_pool(name="const", bufs=1) as const_pool,
            tc.tile_pool(name="xin", bufs=4) as xpool,
            tc.tile_pool(name="out", bufs=3) as opool,
            tc.tile_pool(name="psum", bufs=4, space=bass.MemorySpace.PSUM) as pspool,
        ):
            w_sb = const_pool.tile([128, K * OUT_C], mm_dt)
            nc.sync.dma_start(w_sb[:], wt[:])
            b_sb = const_pool.tile([64, N_SH], f32)
            nc.sync.dma_start(b_sb[:], bias[:])

            # full-width tiles for pairs 0-6; split pair 7 into halves so the
            # pipeline tail (last tile's compute + store) is short
            jobs = [(p, 0, D_SH) for p in range(7)]
            jobs += [(7, 0, D_SH // 2), (7, D_SH // 2, D_SH)]
            for p, d0, d1 in jobs:
                dw = d1 - d0
                nw = dw // K
                xt = xpool.tile([128, dw], mm_dt, tag="xt")
                nc.sync.dma_start(
                    xt[:],
                    x[2 * p : 2 * p + 2, :, d0:d1].rearrange("b c d -> (b c) d"),
                )
                # [128, k, n] strided view of the patch layout
                xr = xt[:].rearrange("p (n k) -> p k n", k=K)
                ot = opool.tile([128, nw], f32, tag="ot")
                for j in range(nw // CHUNK):
                    # one psum bank per batch half; fp32r matmuls must write
                    # PSUM at partition 0, so the halves use PE row-tiles
                    # (0,0)/(64,0) into separate banks
                    pss = [
                        pspool.tile([64, CHUNK], f32, name=f"ps{h}", tag=f"ps{h}")
                        for h in (0, 1)
                    ]
                    for k in range(K):
                        for h in (0, 1):
                            sl = slice(64 * h, 64 * (h + 1))
                            nc.tensor.matmul(
                                pss[h][:, :],
                                w_sb[sl, 64 * k : 64 * (k + 1)],
                                xr[sl, k, CHUNK * j : CHUNK * (j + 1)],
                                start=(k == 0),
                                stop=(k == K - 1),
                            )
                    boff = d0 // K + CHUNK * j
                    for h in (0, 1):
                        sl = slice(64 * h, 64 * (h + 1))
                        nc.vector.tensor_add(
                            ot[sl, CHUNK * j : CHUNK * (j + 1)],
                            pss[h][:, :],
                            b_sb[:, boff : boff + CHUNK],
                        )
                nc.sync.dma_start(
                    y[2 * p : 2 * p + 2, :, d0 // K : d1 // K].rearrange(
                        "b c d -> (b c) d"
                    ),
                    ot[:],
                )
    nc.compile()
    return nc


def get_module():
    if "nc" not in _CACHE:
        _CACHE["nc"] = _build_module()
    return _CACHE["nc"]


def make_in_maps(x, weight, bias):
    """Host-side prep: shard x/bias over d, pre-transpose+scale weight."""
    x = np.asarray(x, dtype=np.float32)
    weight = np.asarray(weight, dtype=np.float32)
    bias = np.asarray(bias, dtype=np.float32)

    # wt[h*64+c, k*64+o] = weight[o, c, 0, k] / 8  (fold 1/sqrt(cin))
    wtb = (weight[:, :, 0, :] / np.sqrt(CIN)).transpose(1, 2, 0).reshape(CIN, K * OUT_C)
    wt = np.empty((128, K * OUT_C), np.float32)
    wt[0:64] = wtb
    wt[64:128] = wtb

    bias_p = (bias[0] * 0.1).astype(np.float32)  # (64, 16384)

    in_maps = []
    for c in range(N_CORES):
        in_maps.append(
            {
                "x": np.ascontiguousarray(x[:, :, c * D_SH : (c + 1) * D_SH]),
                "wt": wt,
                "bias": np.ascontiguousarray(bias_p[:, c * N_SH : (c + 1) * N_SH]),
            }
        )
    return in_maps


def run(x, weight, bias, **spmd_kwargs):
    """Run on 8 cores; returns (full y, BassKernelResults)."""
    from concourse import bass_utils

    nc = get_module()
    in_maps = make_in_maps(x, weight, bias)
    res = bass_utils.run_bass_kernel_spmd(
        nc, in_maps, core_ids=list(range(N_CORES)), **spmd_kwargs
    )
    y = np.empty((BS, OUT_C, OUT_DIM), np.float32)
    for c in range(N_CORES):
        y[:, :, c * N_SH : (c + 1) * N_SH] = res.results[c]["y"]
    return y, res


def kernel(x, weight, bias):
    y, _ = run(x, weight, bias)
    return y
